# revision 14
# baseline (speedup 1.0000x reference)
"""Multi-head attention (B=4, S=2048, E=1024, H=16, D=64) on 8 Trainium2 cores.

Sharding: batch x head-group. Core c handles batch c//2 and heads
(c%2)*8 .. (c%2)*8+7. Each core computes its QKV projection slice, the
attention for its 8 heads, and a partial output projection; the host sums
the two partials per batch and adds out_b (plus the v-bias correction
bv @ wo^T, folded out of the device kernel).

Device dataflow (per core), attention math in transposed layout, bf16
matmul operands, fp32 PSUM accumulation:
  v:  v [t, hd] = x Wv^T (no bias; folded into host-side out_b correction)
  qk: qkT [1024, S] = Wqk x^T + bqk (q rows pre-scaled by 1/sqrt(D));
      head-group 0 first, later groups interleaved into the attention loop.
  attention per (head, s-block): S^T[t,s] = kT.T qT -> exp (Act engine) ->
      AV with ones-augmented v gives out^T [64, s] + softmax denominator;
      reciprocal (DVE) -> partition-broadcast via K=1 matmul -> normalize.
  out: y_part [S, 1024] = aT.T @ woT, streamed to DRAM.

Pipelining: expT/av double-buffered so scores+exp of pair p+1 overlap the
AV matmuls of pair p; qk projection for head-group g+1 interleaved into
group g's pairs so the Act engine starts early and never starves.
"""

from contextlib import ExitStack

import ml_dtypes
import numpy as np

import concourse.bacc as bacc
import concourse.bass as bass
import concourse.mybir as mybir
import concourse.tile as tile
from concourse import bass_utils

B, S, E, H, D = 4, 2048, 1024, 16, 64
NCORES = 8
HC = H // 2          # heads per core
DV = HC * D          # v width per core (= out-proj contraction per core)
EO = E               # out-proj output width
SB = 512             # s-block (matmul moving free dim)
TGRP = 2             # t-chunks per exp group (psum banks per scores tile)
EC = E // 128        # e-chunks (contraction tiles for projections)
MQK = 2 * HC * D // 128  # qk chunks (first half q, second half k)
DVC = DV // 128      # aT partition chunks
TC = S // 128        # t-chunks
NSB = S // SB        # s-blocks

F32 = mybir.dt.float32
F32R = mybir.dt.float32r
BF16 = mybir.dt.bfloat16

USE_FP8_AV = False   # fp8e4 DoubleRow AV: 2x PE throughput but rel err ~3e-2 — too lossy

MM_DT = BF16         # projection matmul operand dtype
P_DT = mybir.dt.float8e4 if USE_FP8_AV else BF16   # exp(scores) dtype (AV rhs)
V_DT = mybir.dt.float8e4 if USE_FP8_AV else BF16   # v dtype (AV lhsT)
NP_FP8 = ml_dtypes.float8_e4m3

NP_BF16 = ml_dtypes.bfloat16


def build_nc():
    assert TC % TGRP == 0 and D == 64 and MQK == 8

    nc = bacc.Bacc("TRN2", target_bir_lowering=False, debug=False,
                   enable_asserts=False, num_devices=NCORES)

    xT_d = nc.dram_tensor("xT", [E, S], MM_DT, kind="ExternalInput").ap()
    wqk_d = nc.dram_tensor("wqk", [MQK, E, 128], MM_DT, kind="ExternalInput").ap()
    bqk_d = nc.dram_tensor("bqk", [MQK, 128], F32, kind="ExternalInput").ap()
    wv_d = nc.dram_tensor("wv", [E, DV], MM_DT, kind="ExternalInput").ap()
    wo_d = nc.dram_tensor("wo", [DV, EO], MM_DT, kind="ExternalInput").ap()
    ones_v_d = nc.dram_tensor("ones_v", [1], V_DT, kind="ExternalInput").ap()
    ones_r_d = nc.dram_tensor("ones_r", [1], F32R, kind="ExternalInput").ap()
    y_d = nc.dram_tensor("y", [S, EO], F32, kind="ExternalOutput").ap()

    with tile.TileContext(nc) as tc, ExitStack() as ctx:
        # ---------------- persistent SBUF tiles ----------------
        pers = ctx.enter_context(tc.tile_pool(name="pers", bufs=1))
        x_sb = pers.tile([128, EC, S], MM_DT)
        qk_sb = pers.tile([128, MQK, S], MM_DT)
        v_sb = pers.tile([128, TC, HC, D + 1], V_DT)
        aT_sb = pers.tile([128, DVC, S], MM_DT)
        wv_sb = pers.tile([128, EC, DV], MM_DT)
        wo_sb = pers.tile([128, DVC, EO], MM_DT)
        bqk_sb = pers.tile([128, MQK], F32)
        ones_sb = pers.tile([128, D], F32R)

        # input DMAs
        for c in range(EC):
            nc.sync.dma_start(out=x_sb[:, c, :], in_=xT_d[c * 128:(c + 1) * 128, :])
        nc.sync.dma_start(out=wv_sb, in_=wv_d.rearrange("(c p) n -> p c n", p=128))
        nc.sync.dma_start(out=wo_sb, in_=wo_d.rearrange("(c p) o -> p c o", p=128))
        nc.sync.dma_start(out=bqk_sb, in_=bqk_d.rearrange("c p -> p c"))
        # ones column of v (for the softmax denominator row of the AV matmul)
        ones_bcast = bass.AP(tensor=ones_v_d.tensor, offset=ones_v_d.offset,
                             ap=[[0, 128], [0, TC * HC], [1, 1]])
        nc.sync.dma_start(
            out=v_sb[:, :, :, D:D + 1].rearrange("p a b c -> p (a b) c"),
            in_=ones_bcast)
        # ones rows for the K=1 broadcast matmul (row 64 is the one used)
        ones_r_bcast = bass.AP(tensor=ones_r_d.tensor, offset=ones_r_d.offset,
                               ap=[[0, 128], [0, D], [1, 1]])
        nc.sync.dma_start(out=ones_sb, in_=ones_r_bcast)

        # ---------------- v projection (own psum scope) ----------------
        with ExitStack() as ctxV:
            psV = ctxV.enter_context(tc.tile_pool(name="psV", bufs=2, space="PSUM"))
            for t in range(TC):
                ps = psV.tile([128, DV], F32, tag="psv")
                for c in range(EC):
                    nc.tensor.matmul(
                        ps, lhsT=x_sb[:, c, t * 128:(t + 1) * 128],
                        rhs=wv_sb[:, c, :],
                        start=(c == 0), stop=(c == EC - 1))
                nc.vector.tensor_copy(
                    out=v_sb[:, t, :, 0:D],
                    in_=ps.rearrange("p (h d) -> p h d", h=HC))

        # ---------------- main section: qk projection + attention ----------------
        with ExitStack() as ctxM:
            pw = ctxM.enter_context(tc.tile_pool(name="pw", bufs=4))
            pexp = ctxM.enter_context(tc.tile_pool(name="pexp", bufs=2))
            prc = ctxM.enter_context(tc.tile_pool(name="prc", bufs=2))
            pbc = ctxM.enter_context(tc.tile_pool(name="pbc", bufs=2))
            pstg = ctxM.enter_context(tc.tile_pool(name="pstg", bufs=2))
            psQK = ctxM.enter_context(tc.tile_pool(name="psQK", bufs=1, space="PSUM"))
            psSC = ctxM.enter_context(tc.tile_pool(name="psSC", bufs=2, space="PSUM"))
            psAV = ctxM.enter_context(tc.tile_pool(name="psAV", bufs=2, space="PSUM"))
            psBC = ctxM.enter_context(tc.tile_pool(name="psBC", bufs=1, space="PSUM"))

            wqk_tiles = {}

            def load_wqk(j):
                w_t = pw.tile([128, EC, 128], MM_DT, tag="wqk")
                nc.sync.dma_start(
                    out=w_t, in_=wqk_d[j].rearrange("(c p) m -> p c m", p=128))
                wqk_tiles[j] = w_t

            def emit_qk_unit(j, sb):
                ps = psQK.tile([128, SB], F32, tag="psqk")
                w_t = wqk_tiles[j]
                for c in range(EC):
                    nc.tensor.matmul(
                        ps, lhsT=w_t[:, c, :],
                        rhs=x_sb[:, c, sb * SB:(sb + 1) * SB],
                        start=(c == 0), stop=(c == EC - 1))
                nc.vector.tensor_scalar_add(
                    out=qk_sb[:, j, sb * SB:(sb + 1) * SB], in0=ps,
                    scalar1=bqk_sb[:, j:j + 1])

            # head-group 0 projection upfront (k chunk then q chunk)
            load_wqk(4)
            load_wqk(0)
            for j in (4, 0):
                for sb in range(NSB):
                    emit_qk_unit(j, sb)

            # chunk prefetch order for groups 1..3. Slots rotate 4-deep:
            # 5,1 load upfront; 6,2 at pairs 4,5 (slot of 4/0, long dead);
            # 7,3 at pairs 12,13 (slot of 5/1, reads done by pair 8).
            chunk_order = [5, 1, 6, 2, 7, 3]
            for j in chunk_order[:2]:
                load_wqk(j)
            load_at_pair = {4: 6, 5: 2, 12: 7, 13: 3}

            prev = None  # (h, sb, expT tile, av tile)

            def emit_av(prev_state, c4):
                h, _sb, expT, av = prev_state
                if USE_FP8_AV:
                    # one DoubleRow matmul covers the two t-chunks of group c4
                    assert TGRP == 2
                    nc.tensor.matmul(
                        av[0:D + 1, :],
                        lhsT=v_sb[:, 2 * c4:2 * c4 + 2, h, :],
                        rhs=expT[:, 2 * c4:2 * c4 + 2, :],
                        start=(c4 == 0), stop=(c4 == TC // TGRP - 1),
                        perf_mode=mybir.MatmulPerfMode.DoubleRow)
                    return
                for t4 in range(TGRP):
                    t = c4 * TGRP + t4
                    nc.tensor.matmul(
                        av[0:D + 1, :], lhsT=v_sb[:, t, h, :],
                        rhs=expT[:, t, :],
                        start=(t == 0), stop=(t == TC - 1))

            def emit_norm(prev_state):
                h, sb, expT, av = prev_state
                g2, odd = h // 2, h % 2
                rc = prc.tile([128, SB], F32R, tag="rc")
                with nc.allow_low_precision(reason="f32r is bit-identical to f32"):
                    nc.vector.reciprocal(out=rc[D:D + 1, :], in_=av[D:D + 1, :])
                bc = psBC.tile([128, SB], F32, tag="bc")
                nc.tensor.matmul(
                    bc[0:D, :], lhsT=ones_sb[D:D + 1, :],
                    rhs=rc[D:D + 1, :], start=True, stop=True)
                # DVE has a single PSUM port: stage the broadcast in SBUF so
                # the multiply reads only one PSUM operand (av).
                bcsb = pbc.tile([128, SB], F32, tag="bcsb")
                nc.vector.tensor_copy(out=bcsb[0:D, :], in_=bc[0:D, :])
                if not odd:
                    nc.vector.tensor_mul(
                        out=aT_sb[0:D, g2, sb * SB:(sb + 1) * SB],
                        in0=av[0:D, :], in1=bcsb[0:D, :])
                else:
                    stg = pstg.tile([128, SB], MM_DT, tag="stg")
                    nc.vector.tensor_mul(
                        out=stg[0:D, :], in0=av[0:D, :], in1=bcsb[0:D, :])
                    nc.sync.dma_start(
                        out=aT_sb[64:128, g2, sb * SB:(sb + 1) * SB],
                        in_=stg[0:D, :])

            for h in range(HC):
                g, odd = h // 2, h % 2
                p0 = odd * 64
                for sb in range(NSB):
                    p = h * NSB + sb
                    expT = pexp.tile([128, TC, SB], P_DT, tag="expT")
                    av = psAV.tile([128, SB], F32, tag="av")
                    for c4 in range(TC // TGRP):
                        sc = psSC.tile([128, TGRP, SB], F32, tag="sc")
                        for t4 in range(TGRP):
                            t = c4 * TGRP + t4
                            nc.tensor.matmul(
                                sc[:, t4, :],
                                lhsT=qk_sb[p0:p0 + 64, MQK // 2 + g, t * 128:(t + 1) * 128],
                                rhs=qk_sb[p0:p0 + 64, g, sb * SB:(sb + 1) * SB],
                                start=True, stop=True)
                        if prev is not None:
                            emit_av(prev, c4)
                        nc.scalar.activation(
                            out=expT[:, c4 * TGRP:(c4 + 1) * TGRP, :], in_=sc,
                            func=mybir.ActivationFunctionType.Exp)
                    if prev is not None:
                        emit_norm(prev)
                    prev = (h, sb, expT, av)
                    # interleave next head-group's qk projection
                    if p in load_at_pair:
                        load_wqk(load_at_pair[p])
                    if g < 3:
                        u = p - 8 * g
                        j = (MQK // 2 + g + 1) if u < NSB else (g + 1)
                        emit_qk_unit(j, u % NSB)

            # drain the last pair
            for c4 in range(TC // TGRP):
                emit_av(prev, c4)
            emit_norm(prev)

        # ---------------- out projection ----------------
        with ExitStack() as ctxC:
            py = ctxC.enter_context(tc.tile_pool(name="py", bufs=3))
            psC = ctxC.enter_context(tc.tile_pool(name="psC", bufs=4, space="PSUM"))
            for st in range(S // 128):
                y_t = py.tile([128, EO], F32, tag="y")
                for ob in range(EO // SB):
                    ps = psC.tile([128, SB], F32, tag="psC")
                    for j in range(DVC):
                        nc.tensor.matmul(
                            ps, lhsT=aT_sb[:, j, st * 128:(st + 1) * 128],
                            rhs=wo_sb[:, j, ob * SB:(ob + 1) * SB],
                            start=(j == 0), stop=(j == DVC - 1))
                    nc.vector.tensor_copy(out=y_t[:, ob * SB:(ob + 1) * SB], in_=ps)
                nc.sync.dma_start(out=y_d[st * 128:(st + 1) * 128, :], in_=y_t)

    nc.compile()
    return nc


_cache: dict = {}


def _get_nc():
    if "nc" not in _cache:
        _cache["nc"] = build_nc()
    return _cache["nc"]


def _shard_inputs(x_q, qkv_w, qkv_b, out_w):
    """Per-core input maps. Core c: batch c//2, head group c%2."""
    alpha = np.float32(D ** -0.5)
    in_maps = []
    for c in range(NCORES):
        b, g2 = c // 2, c % 2
        hlo = g2 * DV
        wq = qkv_w[hlo:hlo + DV] * alpha
        wk = qkv_w[E + hlo:E + hlo + DV]
        wqk_rows = np.concatenate([wq, wk], axis=0)          # [2*DV, E]
        wqk = np.ascontiguousarray(
            wqk_rows.reshape(MQK, 128, E).transpose(0, 2, 1)).astype(NP_BF16)
        bq = qkv_b[hlo:hlo + DV] * alpha
        bk = qkv_b[E + hlo:E + hlo + DV]
        bqk = np.ascontiguousarray(
            np.concatenate([bq, bk]).reshape(MQK, 128)).astype(np.float32)
        wv = np.ascontiguousarray(
            qkv_w[2 * E + hlo:2 * E + hlo + DV].T).astype(NP_BF16)  # [E, DV]
        wo = np.ascontiguousarray(
            out_w[:, hlo:hlo + DV].T).astype(NP_BF16)               # [DV, EO]
        xT = np.ascontiguousarray(x_q[b].T).astype(NP_BF16)         # [E, S]
        in_maps.append({
            "xT": xT, "wqk": wqk, "bqk": bqk, "wv": wv, "wo": wo,
            "ones_v": np.ones((1,), NP_FP8 if USE_FP8_AV else NP_BF16),
            "ones_r": np.ones((1,), np.float32),
        })
    return in_maps


def kernel(x_q, qkv_w, qkv_b, out_w, out_b):
    import os
    os.environ["BASS_NEVER_TRACE"] = "1"  # axon NTFF hook module is absent here
    x_q = np.asarray(x_q, dtype=np.float32)
    qkv_w = np.asarray(qkv_w, dtype=np.float32)
    qkv_b = np.asarray(qkv_b, dtype=np.float32)
    out_w = np.asarray(out_w, dtype=np.float32)
    out_b = np.asarray(out_b, dtype=np.float32)

    nc = _get_nc()
    in_maps = _shard_inputs(x_q, qkv_w, qkv_b, out_w)
    res = bass_utils.run_bass_kernel_spmd(nc, in_maps, core_ids=list(range(NCORES)))
    parts = [r["y"] for r in res.results]
    # v-bias correction folded out of the device kernel:
    # out += bv  =>  y += bv_slice @ out_w[:, slice].T  (per head-group slice)
    bv_corr = []
    for g2 in range(2):
        hlo = g2 * DV
        bv = qkv_b[2 * E + hlo:2 * E + hlo + DV]
        bv_corr.append(bv @ out_w[:, hlo:hlo + DV].T)
    y = np.empty((B, S, E), dtype=np.float32)
    for b in range(B):
        y[b] = (parts[2 * b] + parts[2 * b + 1]
                + out_b + bv_corr[0] + bv_corr[1])
    return y


# revision 20
# speedup vs baseline: 1.2510x; 1.2510x over previous
"""Multi-head attention (B=4, S=2048, E=1024, H=16, D=64) on 8 Trainium2 cores.

Sharding: batch x head-group. Core c handles batch c//2 and heads
(c%2)*8 .. (c%2)*8+7. Each core computes its QKV projection slice, the
attention for its 8 heads, and a partial output projection; the host sums
the two partials per batch and adds out_b (plus the v-bias correction
bv @ wo^T, folded out of the device kernel).

Device dataflow (per core), attention math in transposed layout, bf16
matmul operands, fp32 PSUM accumulation:
  v:  v [t, hd] = x Wv^T (no bias; folded into host-side out_b correction)
  qk: qkT [1024, S] = Wqk x^T + bqk (q rows pre-scaled by 1/sqrt(D));
      head-group 0 first, later groups interleaved into the attention loop.
  attention per (head, s-block): S^T[t,s] = kT.T qT -> exp (Act engine) ->
      AV with ones-augmented v gives out^T [64, s] + softmax denominator;
      reciprocal (DVE) -> partition-broadcast via K=1 matmul -> normalize.
  out: y_part [S, 1024] = aT.T @ woT, streamed to DRAM.

Pipelining: expT/av double-buffered so scores+exp of pair p+1 overlap the
AV matmuls of pair p; qk projection for head-group g+1 interleaved into
group g's pairs so the Act engine starts early and never starves.
"""

from contextlib import ExitStack

import ml_dtypes
import numpy as np

import concourse.bacc as bacc
import concourse.bass as bass
import concourse.mybir as mybir
import concourse.tile as tile
from concourse import bass_utils

B, S, E, H, D = 4, 2048, 1024, 16, 64
NCORES = 8
HC = H // 2          # heads per core
DV = HC * D          # v width per core (= out-proj contraction per core)
EO = E               # out-proj output width
SB = 512             # s-block (matmul moving free dim)
TGRP = 2             # t-chunks per exp group (psum banks per scores tile)
EC = E // 128        # e-chunks (contraction tiles for projections)
MQK = 2 * HC * D // 128  # qk chunks (first half q, second half k)
DVC = DV // 128      # aT partition chunks
TC = S // 128        # t-chunks
NSB = S // SB        # s-blocks

F32 = mybir.dt.float32
F32R = mybir.dt.float32r
BF16 = mybir.dt.bfloat16

USE_FP8_AV = False   # fp8e4 DoubleRow AV: 2x PE throughput but rel err ~3e-2 — too lossy

MM_DT = BF16         # projection matmul operand dtype
P_DT = mybir.dt.float8e4 if USE_FP8_AV else BF16   # exp(scores) dtype (AV rhs)
V_DT = mybir.dt.float8e4 if USE_FP8_AV else BF16   # v dtype (AV lhsT)
NP_FP8 = ml_dtypes.float8_e4m3

NP_BF16 = ml_dtypes.bfloat16


def build_nc():
    assert TC % TGRP == 0 and D == 64 and MQK == 8

    nc = bacc.Bacc("TRN2", target_bir_lowering=False, debug=False,
                   enable_asserts=False, num_devices=NCORES)

    xT_d = nc.dram_tensor("xT", [E, S], MM_DT, kind="ExternalInput").ap()
    wqk_d = nc.dram_tensor("wqk", [MQK, E, 128], MM_DT, kind="ExternalInput").ap()
    bqk_d = nc.dram_tensor("bqk", [MQK, 128], F32, kind="ExternalInput").ap()
    wv_d = nc.dram_tensor("wv", [E, DV], MM_DT, kind="ExternalInput").ap()
    wo_d = nc.dram_tensor("wo", [DV, EO], MM_DT, kind="ExternalInput").ap()
    ones_v_d = nc.dram_tensor("ones_v", [1], V_DT, kind="ExternalInput").ap()
    ones_r_d = nc.dram_tensor("ones_r", [1], F32R, kind="ExternalInput").ap()
    y_d = nc.dram_tensor("y", [S, EO], F32, kind="ExternalOutput").ap()

    with tile.TileContext(nc) as tc, ExitStack() as ctx:
        # ---------------- persistent SBUF tiles ----------------
        pers = ctx.enter_context(tc.tile_pool(name="pers", bufs=1))
        x_sb = pers.tile([128, EC, S], MM_DT)
        qk_sb = pers.tile([128, MQK, S], MM_DT)
        v_sb = pers.tile([128, TC, HC, D + 1], V_DT)
        aT_sb = pers.tile([128, DVC, S], MM_DT)
        wv_sb = pers.tile([128, EC, DV], MM_DT)
        wo_sb = pers.tile([128, DVC, EO], MM_DT)
        bqk_sb = pers.tile([128, MQK], F32)
        ones_sb = pers.tile([128, D], F32R)

        # input DMAs
        for c in range(EC):
            nc.sync.dma_start(out=x_sb[:, c, :], in_=xT_d[c * 128:(c + 1) * 128, :])
        nc.sync.dma_start(out=wv_sb, in_=wv_d.rearrange("(c p) n -> p c n", p=128))
        nc.sync.dma_start(out=bqk_sb, in_=bqk_d.rearrange("c p -> p c"))
        # ones column of v (for the softmax denominator row of the AV matmul)
        ones_bcast = bass.AP(tensor=ones_v_d.tensor, offset=ones_v_d.offset,
                             ap=[[0, 128], [0, TC * HC], [1, 1]])
        nc.sync.dma_start(
            out=v_sb[:, :, :, D:D + 1].rearrange("p a b c -> p (a b) c"),
            in_=ones_bcast)
        # ones rows for the K=1 broadcast matmul (row 64 is the one used)
        ones_r_bcast = bass.AP(tensor=ones_r_d.tensor, offset=ones_r_d.offset,
                               ap=[[0, 128], [0, D], [1, 1]])
        nc.sync.dma_start(out=ones_sb, in_=ones_r_bcast)

        # ---------------- v projection + head-group-0 qk (own psum scope) ----
        wqk_tiles = {}
        pw0 = ctx.enter_context(tc.tile_pool(name="pw0", bufs=4))

        def load_wqk(j):
            w_t = pw0.tile([128, EC, 128], MM_DT, tag="wqk")
            nc.sync.dma_start(
                out=w_t, in_=wqk_d[j].rearrange("(c p) m -> p c m", p=128))
            wqk_tiles[j] = w_t

        def emit_qk_unit_pool(pool, j, sb):
            ps = pool.tile([128, SB], F32, tag="psqk")
            w_t = wqk_tiles[j]
            for c in range(EC):
                nc.tensor.matmul(
                    ps, lhsT=w_t[:, c, :],
                    rhs=x_sb[:, c, sb * SB:(sb + 1) * SB],
                    start=(c == 0), stop=(c == EC - 1))
            nc.vector.tensor_scalar_add(
                out=qk_sb[:, j, sb * SB:(sb + 1) * SB], in0=ps,
                scalar1=bqk_sb[:, j:j + 1])

        load_wqk(4)
        load_wqk(0)
        with ExitStack() as ctxV:
            psV = ctxV.enter_context(tc.tile_pool(name="psV", bufs=2, space="PSUM"))
            psQ0 = ctxV.enter_context(tc.tile_pool(name="psQ0", bufs=2, space="PSUM"))
            for t in range(TC):
                ps = psV.tile([128, DV], F32, tag="psv")
                for c in range(EC):
                    nc.tensor.matmul(
                        ps, lhsT=x_sb[:, c, t * 128:(t + 1) * 128],
                        rhs=wv_sb[:, c, :],
                        start=(c == 0), stop=(c == EC - 1))
                nc.vector.tensor_copy(
                    out=v_sb[:, t, :, 0:D],
                    in_=ps.rearrange("p (h d) -> p h d", h=HC))
            # head-group 0 projection here: double-buffered psum, banks free
            for j in (4, 0):
                for sb in range(NSB):
                    emit_qk_unit_pool(psQ0, j, sb)

        # ---------------- main section: qk projection + attention ----------------
        with ExitStack() as ctxM:
            pexp = ctxM.enter_context(tc.tile_pool(name="pexp", bufs=2))
            prc = ctxM.enter_context(tc.tile_pool(name="prc", bufs=2))
            pbc = ctxM.enter_context(tc.tile_pool(name="pbc", bufs=2))
            pstg = ctxM.enter_context(tc.tile_pool(name="pstg", bufs=2))
            psQK = ctxM.enter_context(tc.tile_pool(name="psQK", bufs=1, space="PSUM"))
            psSC = ctxM.enter_context(tc.tile_pool(name="psSC", bufs=2, space="PSUM"))
            psAV = ctxM.enter_context(tc.tile_pool(name="psAV", bufs=2, space="PSUM"))
            psBC = ctxM.enter_context(tc.tile_pool(name="psBC", bufs=1, space="PSUM"))

            def emit_qk_unit(j, sb):
                emit_qk_unit_pool(psQK, j, sb)

            # chunk prefetch order for groups 1..3. Slots rotate 4-deep:
            # 5,1 load upfront; 6,2 at pairs 4,5 (slot of 4/0, long dead);
            # 7,3 at pairs 12,13 (slot of 5/1, reads done by pair 8).
            chunk_order = [5, 1, 6, 2, 7, 3]
            for j in chunk_order[:2]:
                load_wqk(j)
            load_at_pair = {4: 6, 5: 2, 12: 7, 13: 3}

            prev = None  # (h, sb, expT tile, av tile)

            def emit_av(prev_state, c4):
                h, _sb, expT, av = prev_state
                if USE_FP8_AV:
                    # one DoubleRow matmul covers the two t-chunks of group c4
                    assert TGRP == 2
                    nc.tensor.matmul(
                        av[0:D + 1, :],
                        lhsT=v_sb[:, 2 * c4:2 * c4 + 2, h, :],
                        rhs=expT[:, 2 * c4:2 * c4 + 2, :],
                        start=(c4 == 0), stop=(c4 == TC // TGRP - 1),
                        perf_mode=mybir.MatmulPerfMode.DoubleRow)
                    return
                for t4 in range(TGRP):
                    t = c4 * TGRP + t4
                    nc.tensor.matmul(
                        av[0:D + 1, :], lhsT=v_sb[:, t, h, :],
                        rhs=expT[:, t, :],
                        start=(t == 0), stop=(t == TC - 1))

            def emit_norm(prev_state):
                h, sb, expT, av = prev_state
                g2, odd = h // 2, h % 2
                rc = prc.tile([128, SB], F32R, tag="rc")
                with nc.allow_low_precision(reason="f32r is bit-identical to f32"):
                    nc.vector.reciprocal(out=rc[D:D + 1, :], in_=av[D:D + 1, :])
                bc = psBC.tile([128, SB], F32, tag="bc")
                nc.tensor.matmul(
                    bc[0:D, :], lhsT=ones_sb[D:D + 1, :],
                    rhs=rc[D:D + 1, :], start=True, stop=True)
                # DVE has a single PSUM port: stage the broadcast in SBUF so
                # the multiply reads only one PSUM operand (av).
                bcsb = pbc.tile([128, SB], F32, tag="bcsb")
                nc.vector.tensor_copy(out=bcsb[0:D, :], in_=bc[0:D, :])
                if not odd:
                    nc.vector.tensor_mul(
                        out=aT_sb[0:D, g2, sb * SB:(sb + 1) * SB],
                        in0=av[0:D, :], in1=bcsb[0:D, :])
                else:
                    stg = pstg.tile([128, SB], MM_DT, tag="stg")
                    nc.vector.tensor_mul(
                        out=stg[0:D, :], in0=av[0:D, :], in1=bcsb[0:D, :])
                    nc.sync.dma_start(
                        out=aT_sb[64:128, g2, sb * SB:(sb + 1) * SB],
                        in_=stg[0:D, :])

            for h in range(HC):
                g, odd = h // 2, h % 2
                p0 = odd * 64
                for sb in range(NSB):
                    p = h * NSB + sb
                    expT = pexp.tile([128, TC, SB], P_DT, tag="expT")
                    av = psAV.tile([128, SB], F32, tag="av")
                    for c4 in range(TC // TGRP):
                        sc = psSC.tile([128, TGRP, SB], F32, tag="sc")
                        for t4 in range(TGRP):
                            t = c4 * TGRP + t4
                            nc.tensor.matmul(
                                sc[:, t4, :],
                                lhsT=qk_sb[p0:p0 + 64, MQK // 2 + g, t * 128:(t + 1) * 128],
                                rhs=qk_sb[p0:p0 + 64, g, sb * SB:(sb + 1) * SB],
                                start=True, stop=True)
                        if prev is not None:
                            emit_av(prev, c4)
                        nc.scalar.activation(
                            out=expT[:, c4 * TGRP:(c4 + 1) * TGRP, :], in_=sc,
                            func=mybir.ActivationFunctionType.Exp)
                    if prev is not None:
                        emit_norm(prev)
                    prev = (h, sb, expT, av)
                    # interleave next head-group's qk projection
                    if p in load_at_pair:
                        load_wqk(load_at_pair[p])
                    if p == 16:  # wo needed only in phase C; DMA idle here
                        nc.sync.dma_start(
                            out=wo_sb,
                            in_=wo_d.rearrange("(c p) o -> p c o", p=128))
                    if g < 3:
                        u = p - 8 * g
                        j = (MQK // 2 + g + 1) if u < NSB else (g + 1)
                        emit_qk_unit(j, u % NSB)

            # drain the last pair
            for c4 in range(TC // TGRP):
                emit_av(prev, c4)
            emit_norm(prev)

        # ---------------- out projection ----------------
        with ExitStack() as ctxC:
            py = ctxC.enter_context(tc.tile_pool(name="py", bufs=3))
            psC = ctxC.enter_context(tc.tile_pool(name="psC", bufs=4, space="PSUM"))
            for st in range(S // 128):
                y_t = py.tile([128, EO], F32, tag="y")
                for ob in range(EO // SB):
                    ps = psC.tile([128, SB], F32, tag="psC")
                    for j in range(DVC):
                        nc.tensor.matmul(
                            ps, lhsT=aT_sb[:, j, st * 128:(st + 1) * 128],
                            rhs=wo_sb[:, j, ob * SB:(ob + 1) * SB],
                            start=(j == 0), stop=(j == DVC - 1))
                    nc.vector.tensor_copy(out=y_t[:, ob * SB:(ob + 1) * SB], in_=ps)
                nc.sync.dma_start(out=y_d[st * 128:(st + 1) * 128, :], in_=y_t)

    nc.compile()
    return nc


_cache: dict = {}


def _get_nc():
    if "nc" not in _cache:
        _cache["nc"] = build_nc()
    return _cache["nc"]


def _shard_inputs(x_q, qkv_w, qkv_b, out_w):
    """Per-core input maps. Core c: batch c//2, head group c%2."""
    alpha = np.float32(D ** -0.5)
    in_maps = []
    for c in range(NCORES):
        b, g2 = c // 2, c % 2
        hlo = g2 * DV
        wq = qkv_w[hlo:hlo + DV] * alpha
        wk = qkv_w[E + hlo:E + hlo + DV]
        wqk_rows = np.concatenate([wq, wk], axis=0)          # [2*DV, E]
        wqk = np.ascontiguousarray(
            wqk_rows.reshape(MQK, 128, E).transpose(0, 2, 1)).astype(NP_BF16)
        bq = qkv_b[hlo:hlo + DV] * alpha
        bk = qkv_b[E + hlo:E + hlo + DV]
        bqk = np.ascontiguousarray(
            np.concatenate([bq, bk]).reshape(MQK, 128)).astype(np.float32)
        wv = np.ascontiguousarray(
            qkv_w[2 * E + hlo:2 * E + hlo + DV].T).astype(NP_BF16)  # [E, DV]
        wo = np.ascontiguousarray(
            out_w[:, hlo:hlo + DV].T).astype(NP_BF16)               # [DV, EO]
        xT = np.ascontiguousarray(x_q[b].T).astype(NP_BF16)         # [E, S]
        in_maps.append({
            "xT": xT, "wqk": wqk, "bqk": bqk, "wv": wv, "wo": wo,
            "ones_v": np.ones((1,), NP_FP8 if USE_FP8_AV else NP_BF16),
            "ones_r": np.ones((1,), np.float32),
        })
    return in_maps


def kernel(x_q, qkv_w, qkv_b, out_w, out_b):
    import os
    os.environ["BASS_NEVER_TRACE"] = "1"  # axon NTFF hook module is absent here
    x_q = np.asarray(x_q, dtype=np.float32)
    qkv_w = np.asarray(qkv_w, dtype=np.float32)
    qkv_b = np.asarray(qkv_b, dtype=np.float32)
    out_w = np.asarray(out_w, dtype=np.float32)
    out_b = np.asarray(out_b, dtype=np.float32)

    nc = _get_nc()
    in_maps = _shard_inputs(x_q, qkv_w, qkv_b, out_w)
    res = bass_utils.run_bass_kernel_spmd(nc, in_maps, core_ids=list(range(NCORES)))
    parts = [r["y"] for r in res.results]
    # v-bias correction folded out of the device kernel:
    # out += bv  =>  y += bv_slice @ out_w[:, slice].T  (per head-group slice)
    bv_corr = []
    for g2 in range(2):
        hlo = g2 * DV
        bv = qkv_b[2 * E + hlo:2 * E + hlo + DV]
        bv_corr.append(bv @ out_w[:, hlo:hlo + DV].T)
    y = np.empty((B, S, E), dtype=np.float32)
    for b in range(B):
        y[b] = (parts[2 * b] + parts[2 * b + 1]
                + out_b + bv_corr[0] + bv_corr[1])
    return y
